# revision 6
# baseline (speedup 1.0000x reference)
"""Trainium2 Bass kernel for nn_Chimera_80934363725826 (gnn_message_passing).

Math: the reference builds a grid-DAG adjacency A (left->right, top->bottom
edges, weights from sigmoid(-(dt+bias)) scaled by 0.95/sqrt(num_incident)),
computes M = (I-A)^{-1} by repeated squaring, and returns y = M @ x + D*x.

Since (I-A) is unit-lower-triangular in raster order with only two sub-
diagonals (-1 and -14), y = (I-A)^{-1} x is exactly the 2D first-order
recurrence
    y[i,j] = x[i,j] + al[i,j]*y[i,j-1] + at[i,j]*y[i-1,j]
over the 14x14 grid (per batch*head, per feature). We solve it with 14
row-wise prefix scans on the vector engine (tensor_tensor_scan), one per
grid row, batched over all 96 local (b,h) pairs (partition dim) and all 64
features (free dim).

Sharding: data-parallel over batch B=32 -> 4 batches/core on 8 cores.
"""

import numpy as np

import concourse.bass as bass
import concourse.bacc as bacc
import concourse.mybir as mybir
from concourse.bass import AP
from concourse.tile import TileContext
from concourse.bass_utils import run_bass_kernel_spmd

F32 = mybir.dt.float32

# problem constants (hardcoded; kernel.py must be self-contained)
HG, WG = 14, 14          # grid
L = HG * WG              # 196 nodes
B, NH, P = 32, 24, 64    # batch, heads, headdim
NCORES = 8
BLOC = B // NCORES       # 4 batches per core
NPART = BLOC * NH        # 96 partitions (b,h) pairs
ROW = P * WG             # 896 elements per grid row per partition
TOT = L * P              # 12544 elements per partition
INVERSE_FACTOR = 0.95

_CACHE = {}


def _host_tables():
    """Constant (input-independent) coefficient scale tables [2, L]."""
    nie = 2.0 * np.ones((HG, WG))
    nie[:, 0] -= 1.0
    nie[0, :] -= 1.0
    nie[nie < 1e-6] = 1.0
    norm = (INVERSE_FACTOR / np.sqrt(nie)).astype(np.float32)
    mask_l = np.ones((HG, WG), np.float32)
    mask_l[:, 0] = 0.0          # no left neighbor in first column
    mask_t = np.ones((HG, WG), np.float32)
    mask_t[0, :] = 0.0          # no top neighbor in first row
    tab = np.concatenate([(norm * mask_l).ravel(), (norm * mask_t).ravel()])
    return tab.astype(np.float32)  # [392]


def _build_program():
    nc = bacc.Bacc("TRN2", target_bir_lowering=False, debug=False,
                   num_devices=NCORES)
    xin = nc.dram_tensor("xin", [NPART, TOT], F32, kind="ExternalInput")
    dtin = nc.dram_tensor("dtin", [NPART, 2 * L], F32, kind="ExternalInput")
    tabin = nc.dram_tensor("tabin", [NPART, 2 * L], F32, kind="ExternalInput")
    nbin = nc.dram_tensor("nbin", [NPART, 1], F32, kind="ExternalInput")
    dpin = nc.dram_tensor("dpin", [NPART, 1], F32, kind="ExternalInput")
    yout = nc.dram_tensor("yout", [NPART, TOT], F32, kind="ExternalOutput")

    x_chunks = [(0, 2), (3, 6), (7, 10), (11, 13)]

    with TileContext(nc) as tc:
        with tc.tile_pool(name="main", bufs=1) as pool, \
             tc.tile_pool(name="rowtmp", bufs=3) as rpool:
            xt = pool.tile([NPART, TOT], F32)     # x, natural (i,j,f) layout
            yt = pool.tile([NPART, TOT], F32)     # y, per-row (i,f,j) layout
            ot = pool.tile([NPART, TOT], F32)     # out, natural layout
            dtt = pool.tile([NPART, 2 * L], F32)
            tab = pool.tile([NPART, 2 * L], F32)
            coeff = pool.tile([NPART, 2 * L], F32)  # al | at, scaled+masked
            nb = pool.tile([NPART, 1], F32)
            dp = pool.tile([NPART, 1], F32)

            # --- input DMAs ---
            nc.sync.dma_start(out=dtt[:, :], in_=dtin[:, :])
            nc.sync.dma_start(out=tab[:, :], in_=tabin[:, :])
            nc.sync.dma_start(out=nb[:, :], in_=nbin[:, :])
            nc.sync.dma_start(out=dp[:, :], in_=dpin[:, :])
            for r0, r1 in x_chunks:
                nc.sync.dma_start(
                    out=xt[:, r0 * ROW:(r1 + 1) * ROW],
                    in_=xin[:, r0 * ROW:(r1 + 1) * ROW],
                )

            # --- coefficients: coeff = sigmoid(-(dt + bias)) * table ---
            sig = pool.tile([NPART, 2 * L], F32)
            nc.scalar.activation(
                out=sig[:, :], in_=dtt[:, :],
                func=mybir.ActivationFunctionType.Sigmoid,
                bias=nb[:, 0:1], scale=-1.0,
            )
            nc.vector.tensor_mul(out=coeff[:, :], in0=sig[:, :], in1=tab[:, :])

            # views
            # natural x: p (i j f); transposed-in-row view: p i f j
            x_fj = xt.rearrange("p (i j f) -> p i f j", i=HG, j=WG, f=P)
            # y stored per-row f-major: p (i f j)
            y_fj = yt.rearrange("p (i f j) -> p i f j", i=HG, f=P, j=WG)

            # --- row recurrence ---
            for i in range(HG):
                # al row i expanded to (f, j) layout for the scan's data0
                alx = rpool.tile([NPART, ROW], F32, tag="alx")
                alx_fj = alx.rearrange("p (f j) -> p f j", f=P, j=WG)
                al_bc = coeff[:, i * WG:(i + 1) * WG].unsqueeze(1) \
                    .broadcast_to([NPART, P, WG])
                nc.scalar.activation(
                    out=alx_fj, in_=al_bc,
                    func=mybir.ActivationFunctionType.Copy,
                )

                bt = rpool.tile([NPART, ROW], F32, tag="bt")
                bt_fj = bt.rearrange("p (f j) -> p f j", f=P, j=WG)
                if i == 0:
                    # b = x row 0 (reordered to (f,j))
                    nc.scalar.activation(
                        out=bt_fj, in_=x_fj[:, 0],
                        func=mybir.ActivationFunctionType.Copy,
                    )
                else:
                    # t = at[i] * y[i-1];  b = t + x[i]
                    tt = rpool.tile([NPART, ROW], F32, tag="tt")
                    tt_fj = tt.rearrange("p (f j) -> p f j", f=P, j=WG)
                    at_bc = coeff[:, L + i * WG:L + (i + 1) * WG].unsqueeze(1) \
                        .broadcast_to([NPART, P, WG])
                    nc.vector.tensor_mul(out=tt_fj, in0=y_fj[:, i - 1], in1=at_bc)
                    nc.vector.tensor_add(out=bt_fj, in0=tt_fj, in1=x_fj[:, i])

                # y[i] = scan_j: state = al*state + b
                nc.vector.tensor_tensor_scan(
                    out=yt[:, i * ROW:(i + 1) * ROW],
                    data0=alx[:, :],
                    data1=bt[:, :],
                    initial=0.0,
                    op0=mybir.AluOpType.mult,
                    op1=mybir.AluOpType.add,
                )

            # --- out = D*x + y  (gpsimd; y read back in natural (j,f) order) ---
            x_nat = xt.rearrange("p (i j f) -> p i j f", i=HG, j=WG, f=P)
            o_nat = ot.rearrange("p (i j f) -> p i j f", i=HG, j=WG, f=P)
            y_jf = yt.rearrange("p (i f j) -> p i j f", i=HG, f=P, j=WG)
            for r0, r1 in x_chunks:
                for i in range(r0, r1 + 1):
                    # ot = D*x (ScalarE, per-partition scale), then += y (Pool)
                    nc.scalar.activation(
                        out=o_nat[:, i], in_=x_nat[:, i],
                        func=mybir.ActivationFunctionType.Copy,
                        scale=dp[:, 0:1],
                    )
                    nc.gpsimd.tensor_add(
                        out=o_nat[:, i], in0=o_nat[:, i], in1=y_jf[:, i],
                    )
                nc.sync.dma_start(
                    out=yout[:, r0 * ROW:(r1 + 1) * ROW],
                    in_=ot[:, r0 * ROW:(r1 + 1) * ROW],
                )

    nc.compile()
    return nc


def _get_program():
    if "nc" not in _CACHE:
        _CACHE["nc"] = _build_program()
    return _CACHE["nc"]


def make_in_maps(dt, dt_bias, x, D):
    """Host-side sharding + derived tables. Returns list of 8 in_maps."""
    dt = np.ascontiguousarray(np.asarray(dt, dtype=np.float32))
    dt_bias = np.asarray(dt_bias, dtype=np.float32)
    x = np.ascontiguousarray(np.asarray(x, dtype=np.float32))
    D = np.asarray(D, dtype=np.float32)

    tab = _host_tables()                                    # [392]
    tabin = np.ascontiguousarray(np.broadcast_to(tab, (NPART, 2 * L)))
    nb = np.ascontiguousarray(
        np.tile(-dt_bias, BLOC).reshape(NPART, 1))
    dp = np.ascontiguousarray(np.tile(D, BLOC).reshape(NPART, 1))

    in_maps = []
    for c in range(NCORES):
        bs = slice(c * BLOC, (c + 1) * BLOC)
        # dt[:, bs]: [2, BLOC, NH, 14, 14] -> (bl, h, dir, l) -> [96, 392]
        dtc = np.ascontiguousarray(
            dt[:, bs].reshape(2, BLOC, NH, L).transpose(1, 2, 0, 3)
            .reshape(NPART, 2 * L))
        xc = np.ascontiguousarray(x[bs].reshape(NPART, TOT))
        in_maps.append({
            "xin": xc,
            "dtin": dtc,
            "tabin": tabin,
            "nbin": nb,
            "dpin": dp,
        })
    return in_maps


def kernel(dt, dt_bias, x, D):
    nc = _get_program()
    in_maps = make_in_maps(dt, dt_bias, x, D)
    res = run_bass_kernel_spmd(nc, in_maps, core_ids=list(range(NCORES)))
    outs = [r["yout"].reshape(BLOC, NH, L, P) for r in res.results]
    return np.ascontiguousarray(np.concatenate(outs, axis=0))
